# revision 1
# baseline (speedup 1.0000x reference)
"""Multi-head attention forward on 8 Trainium2 NeuronCores.

Problem: x[4,2048,1024], 16 heads (d=64), fp32. out = softmax(QK^T/sqrt(d) + mask) V @ Wo.

Sharding: core = (batch b in 0..3) x (head-group hg in 0..1). Each core handles one
batch element and 8 heads (a 512-wide slice of the model dim). Each core emits a
partial output [2048,1024] (its heads' contribution through Wo); the host sums the
two head-group partials per batch element.

All matmuls run in float32r (full-rate fp32 on the PE for moving dim >= 256). The
BIR verifier requires fp32r matmul operands to be *typed* fp32r at their producing
instruction, so every producer (DMA loads, DVE copies, ACT exp) writes through a
bitcast AP.

Per-core pipeline:
  A) QT,KT = projections in transposed layout [512,2048] (heads pair-packed along
     partitions); V in natural layout, head-interleaved with a ones column per head
     ([128, 8*65]) so the attention matmul also produces the softmax row sums.
  B) per head-pair (2m, 2m+1), per 512-query tile, per 128-key chunk: two K=64
     logits matmuls land in the two halves of a [128,1024] PSUM tile (the two
     heads sit in PE row-groups 0-1/2-3 and run concurrently); one ACT exp over
     [128,1024] with the key mask as per-partition bias and 1/sqrt(d) folded into
     the activation scale; two PT @ V_aug accumulations -> attn_aug[65,512] per
     head (row 64 = exp row sum). Then per head: DVE reciprocal of the row-sum
     row, K=1 outer-product matmul broadcasts it over 64 partitions, DVE multiply
     writes normalized attn^T into SBUF.
  C) out[q,:] = attnT-chunk^T @ Wo-slice (K=512 accumulation), DVE copy, DMA out.
"""
import sys

sys.path.insert(0, "/opt/trn_rl_repo")

import numpy as np

import concourse.bass as bass
import concourse.tile as tile
from concourse import mybir
from concourse.bass_utils import run_bass_kernel_spmd
from concourse.vector_clock import ScopedClock

_wsplit_ctr = [0]


def split_multi_waits(nc):
    """The walrus build in this container accepts at most ONE sync wait per
    instruction. Split any instruction carrying N>1 waits into (N-1)
    single-wait nops on the same engine immediately before it; the original
    instruction keeps one wait and all its updates."""
    for f in nc.m.functions:
        for bb in f.blocks:
            out = []
            changed = False
            for inst in bb.instructions:
                si = inst.sync_info
                waits = list(si.on_wait) if si is not None and si.on_wait else []
                if len(waits) > 1:
                    updates = list(si.on_update) if si.on_update else []
                    for w in waits[1:]:
                        _wsplit_ctr[0] += 1
                        nop = mybir.InstNoOp(
                            name=f"I-wsplit-{_wsplit_ctr[0]}", ins=[], outs=[]
                        )
                        nop.engine = inst.engine
                        nop.sync_info = mybir.SyncInfo(on_wait=[w], on_update=[])
                        out.append(nop)
                    inst.sync_info = mybir.SyncInfo(on_wait=[waits[0]], on_update=updates)
                    changed = True
                out.append(inst)
            if changed:
                bb.instructions = out
    return nc

B, S, D, H, DH = 4, 2048, 1024, 16, 64
HG = 2  # head groups (tensor-parallel)
LD = D // HG  # 512 local model-dim slice
LH = H // HG  # 8 local heads
N_CORES = B * HG
SCALE = float(DH) ** -0.5
NEG_INF = -1e30

FP = mybir.dt.float32
FPR = mybir.dt.float32r
BF = mybir.dt.bfloat16

KC = D // 128  # 8 contraction chunks (projections)
MC = LD // 128  # 4 row chunks of the local dim (= head pairs)
SC = S // 128  # 16 seq chunks of 128
QT = S // 512  # 4 query tiles of 512
Exp = mybir.ActivationFunctionType.Exp
E1 = DH + 1  # per-head V stride incl. ones column


def _fr(ap):
    return ap.bitcast(FPR)


class SplitDrainTileContext(tile.TileContext):
    """The walrus build in this container rejects a Drain instruction with
    more than one sync wait; gate the tail drain with single-wait nops."""

    def _drain_and_barrier(self, tick_clock, wait_clock):
        nc = self.nc
        probe = nc.sync.nop()
        wait_clock.add_sem_waits(
            probe.ins, ScopedClock({None: tick_clock.global_clock})
        )
        si = probe.ins.sync_info
        waits = list(si.on_wait) if si is not None and si.on_wait else []
        updates = list(si.on_update) if si is not None and si.on_update else []
        if len(waits) > 1:
            probe.ins.sync_info = mybir.SyncInfo(on_wait=[waits[0]], on_update=updates)
            for w in waits[1:]:
                n2 = nc.sync.nop()
                n2.ins.sync_info = mybir.SyncInfo(on_wait=[w], on_update=[])
        nc.sync.drain()
        nc.all_engine_barrier()
        popped = nc._tile_sem_poison_stack.pop()
        assert popped is self._sem_poison
        nc.clear_and_free_semaphores(list(self.sems.allocated().values()))
        nc.all_engine_barrier()


def build_nc(for_hw=True):
    nc = bass.Bass(trn_type="TRN2")
    xT = nc.dram_tensor("xT", [D, S], FP, kind="ExternalInput").ap()
    wq = nc.dram_tensor("wq", [D, LD], FP, kind="ExternalInput").ap()
    wk = nc.dram_tensor("wk", [D, LD], FP, kind="ExternalInput").ap()
    wv = nc.dram_tensor("wv", [D, LD], FP, kind="ExternalInput").ap()
    wo = nc.dram_tensor("wo", [LD, D], FP, kind="ExternalInput").ap()
    kbias = nc.dram_tensor("kbias", [128, SC], FP, kind="ExternalInput").ap()
    ones_d = nc.dram_tensor("ones_d", [1, 64], FP, kind="ExternalInput").ap()
    ones_b = nc.dram_tensor("ones_b", [128, LH], BF, kind="ExternalInput").ap()
    out = nc.dram_tensor("out", [S, D], FP, kind="ExternalOutput").ap()

    with SplitDrainTileContext(nc) as tc:
        _body(tc, xT, wq, wk, wv, wo, kbias, ones_d, ones_b, out)
    if for_hw:
        split_multi_waits(nc)
    return nc


def _body(tc, xT, wq, wk, wv, wo, kbias, ones_d, ones_b, out):
    nc = tc.nc
    with (
        tc.tile_pool(name="pers", bufs=1) as pers,
        tc.tile_pool(name="pt", bufs=3) as pt_pool,
        tc.tile_pool(name="rs", bufs=2) as rs_pool,
        tc.tile_pool(name="ot", bufs=4) as ot_pool,
        tc.tile_pool(name="psmm", bufs=1, space="PSUM") as psmm,
    ):
        qt = [pers.tile([128, S], FP, tag=f"qt{m}", name=f"qt{m}") for m in range(MC)]
        kt = [pers.tile([128, S], FP, tag=f"kt{m}", name=f"kt{m}") for m in range(MC)]
        vt = [pers.tile([128, LH * E1], BF, tag=f"v{s}", name=f"v{s}") for s in range(SC)]
        att = [pers.tile([128, S], FP, tag=f"at{m}", name=f"at{m}") for m in range(MC)]
        biasT = pers.tile([128, SC], FP, tag="biasT")
        ones64 = pers.tile([1, 64], FP, tag="ones64")

        nc.sync.dma_start(biasT[:], kbias[:])
        nc.sync.dma_start(_fr(ones64[:]), _fr(ones_d[:]))
        for s in range(SC):
            # fill each head's ones column of V_aug straight from DRAM
            dst = vt[s][:].rearrange("p (h e) -> p h e", e=E1)[:, :, DH : DH + 1]
            nc.sync.dma_start(dst, ones_b[:, 0:LH].unsqueeze(2))

        # ---- stage A: projections ----
        with (
            tc.tile_pool(name="xt", bufs=1) as xt_pool,
            tc.tile_pool(name="w", bufs=1) as w_pool,
        ):
            def load_w(wdram, cast=True):
                wts = [
                    w_pool.tile([128, LD], FP, tag=f"w{k}", name=f"w{k}")
                    for k in range(KC)
                ]
                for k in range(KC):
                    src = wdram[k * 128 : (k + 1) * 128, :]
                    nc.sync.dma_start(_fr(wts[k][:]), _fr(src))
                return wts

            def v_pass(xts, half, wts):
                for sc in range(SC // 2):
                    s_idx = half * (SC // 2) + sc
                    ps = psmm.tile([128, 512], FP, tag="ps", name="ps", bufs=2)
                    j, off = sc // 4, (sc % 4) * 128
                    for k in range(KC):
                        nc.tensor.matmul(
                            ps[:],
                            _fr(xts[k][j][:, off : off + 128]),
                            _fr(wts[k][:]),
                            start=(k == 0),
                            stop=(k == KC - 1),
                        )
                    src = ps[:].rearrange("p (h e) -> p h e", h=LH)
                    dst = vt[s_idx][:].rearrange("p (h e) -> p h e", e=E1)[:, :, 0:DH]
                    nc.vector.tensor_copy(dst, src)

            def qk_pass(xts, half, wts, dstT, ms):
                for m in ms:
                    for q2 in range(2):
                        ps = psmm.tile([128, 512], FP, tag="ps", name="ps", bufs=2)
                        for k in range(KC):
                            nc.tensor.matmul(
                                ps[:],
                                _fr(wts[k][:, m * 128 : (m + 1) * 128]),
                                _fr(xts[k][q2][:]),
                                start=(k == 0),
                                stop=(k == KC - 1),
                            )
                        qlo = half * 1024 + q2 * 512
                        nc.vector.tensor_copy(_fr(dstT[m][:, qlo : qlo + 512]), ps[:])

            for half in range(2):
                # two 512-col slices per contraction chunk: the first V matmul
                # needs only the j=0 slices (2MB) instead of the full half (4MB)
                xts = [
                    [
                        xt_pool.tile([128, 512], FP, tag=f"xt{k}_{j}", name=f"xt{k}_{j}")
                        for j in range(2)
                    ]
                    for k in range(KC)
                ]
                for j in range(2):
                    for k in range(KC):
                        lo = half * 1024 + j * 512
                        nc.sync.dma_start(
                            _fr(xts[k][j][:]),
                            _fr(xT[k * 128 : (k + 1) * 128, lo : lo + 512]),
                        )
                if half == 0:
                    # V first (stage B's AV loop hits half-1 V chunks first)
                    v_pass(xts, half, load_w(wv))
                    qk_pass(xts, half, load_w(wq), qt, range(MC))
                    qk_pass(xts, half, load_w(wk), kt, range(MC))
                else:
                    # finish pair m=0 first so stage B starts while A finishes
                    wq_t = load_w(wq)
                    qk_pass(xts, half, wq_t, qt, [0])
                    wk_t = load_w(wk)
                    qk_pass(xts, half, wk_t, kt, [0])
                    v_pass(xts, half, load_w(wv))
                    qk_pass(xts, half, load_w(wq), qt, [1, 2, 3])
                    qk_pass(xts, half, load_w(wk), kt, [1, 2, 3])

        # ---- stages B+C ----
        with tc.tile_pool(name="wo", bufs=1) as wo_pool:
            wos = [
                wo_pool.tile([128, D], FP, tag=f"wo{j}", name=f"wo{j}")
                for j in range(MC)
            ]
            for j in range(MC):
                nc.sync.dma_start(_fr(wos[j][:]), _fr(wo[j * 128 : (j + 1) * 128, :]))

            def stage_c_slab(q):
                # output projection for one 512-query slab (4 chunks of 128)
                for qc in range(4 * q, 4 * (q + 1)):
                    for n in range(2):
                        ps = psmm.tile([128, 512], FP, tag="ps", name="psc", bufs=2)
                        for j in range(MC):
                            nc.tensor.matmul(
                                ps[:],
                                _fr(att[j][:, qc * 128 : (qc + 1) * 128]),
                                _fr(wos[j][:, n * 512 : (n + 1) * 512]),
                                start=(j == 0),
                                stop=(j == MC - 1),
                            )
                        ot = ot_pool.tile([128, 512], FP, tag="ot", name="ot")
                        nc.vector.tensor_copy(ot[:], ps[:])
                        nc.sync.dma_start(
                            out[qc * 128 : (qc + 1) * 128, n * 512 : (n + 1) * 512],
                            ot[:],
                        )

            # stage B: attention, one head-pair at a time
            for m in range(MC):
                hA, hB = 2 * m, 2 * m + 1
                for q in range(QT):
                    qs = slice(q * 512, (q + 1) * 512)
                    aA = psmm.tile([128, 512], FP, tag="aA", name="aA")
                    aB = psmm.tile([128, 512], FP, tag="aB", name="aB")
                    for kc in range(SC):
                        ks = slice(kc * 128, (kc + 1) * 128)
                        lg = psmm.tile([128, 1024], FP, tag="lg", name="lg", bufs=2)
                        nc.tensor.matmul(
                            lg[:, 0:512],
                            _fr(kt[m][0:64, ks]),
                            _fr(qt[m][0:64, qs]),
                            start=True,
                            stop=True,
                        )
                        nc.tensor.matmul(
                            lg[:, 512:1024],
                            _fr(kt[m][64:128, ks]),
                            _fr(qt[m][64:128, qs]),
                            start=True,
                            stop=True,
                        )
                        pt = pt_pool.tile([128, 1024], BF, tag="pt", name="pt")
                        nc.scalar.activation(
                            pt[:], lg[:], Exp, bias=biasT[:, kc : kc + 1], scale=SCALE
                        )
                        nc.tensor.matmul(
                            aA[0:65, :],
                            vt[kc][:, hA * E1 : (hA + 1) * E1],
                            pt[:, 0:512],
                            start=(kc == 0),
                            stop=(kc == SC - 1),
                            skip_group_check=True,
                        )
                        nc.tensor.matmul(
                            aB[0:65, :],
                            vt[kc][:, hB * E1 : (hB + 1) * E1],
                            pt[:, 512:1024],
                            start=(kc == 0),
                            stop=(kc == SC - 1),
                            skip_group_check=True,
                        )
                    for po, a_ps in ((0, aA), (64, aB)):
                        rs = rs_pool.tile([1, 512], FP, tag="rs", name="rs")
                        with nc.allow_low_precision(reason="fp32r operand typing"):
                            nc.vector.reciprocal(_fr(rs[:]), a_ps[64:65, :])
                        bc = psmm.tile([64, 512], FP, tag="ps", name="bc", bufs=2)
                        nc.tensor.matmul(
                            bc[:], _fr(ones64[:]), _fr(rs[:]), start=True, stop=True
                        )
                        bcs = rs_pool.tile([64, 512], FP, tag="bcs", name="bcs", bufs=2)
                        nc.vector.tensor_copy(bcs[:], bc[:])
                        nc.vector.tensor_tensor(
                            out=_fr(att[m][po : po + 64, qs]),
                            in0=a_ps[0:64, :],
                            in1=bcs[:],
                            op=mybir.AluOpType.mult,
                        )
                    if m == MC - 1:
                        stage_c_slab(q)



_nc = None


def get_nc():
    global _nc
    if _nc is None:
        _nc = build_nc()
    return _nc


def make_in_maps(x, mask, Wq, Wk, Wv, Wo):
    x = np.asarray(x, dtype=np.float32)
    mask = np.asarray(mask)
    Wq, Wk, Wv, Wo = (np.asarray(w, dtype=np.float32) for w in (Wq, Wk, Wv, Wo))
    in_maps = []
    for c in range(N_CORES):
        b, hg = c // HG, c % HG
        lo, hi = hg * LD, (hg + 1) * LD
        kb = np.where(mask[b], 0.0, NEG_INF).astype(np.float32)
        in_maps.append(
            {
                "xT": np.ascontiguousarray(x[b].T),
                "wq": np.ascontiguousarray(Wq[:, lo:hi]),
                "wk": np.ascontiguousarray(Wk[:, lo:hi]),
                "wv": np.ascontiguousarray(Wv[:, lo:hi]),
                "wo": np.ascontiguousarray(Wo[lo:hi, :]),
                "kbias": np.ascontiguousarray(kb.reshape(SC, 128).T),
                "ones_d": np.ones((1, 64), np.float32),
                "ones_b": np.ones((128, LH), np.float32).astype(__import__("ml_dtypes").bfloat16),
            }
        )
    return in_maps


def kernel(x, mask, Wq, Wk, Wv, Wo):
    nc = get_nc()
    in_maps = make_in_maps(x, mask, Wq, Wk, Wv, Wo)
    res = run_bass_kernel_spmd(nc, in_maps, list(range(N_CORES)))
    outs = np.empty((B, S, D), dtype=np.float32)
    for b in range(B):
        outs[b] = res.results[2 * b]["out"] + res.results[2 * b + 1]["out"]
    return outs



# revision 22
# speedup vs baseline: 1.3918x; 1.3918x over previous
"""Multi-head attention forward on 8 Trainium2 NeuronCores.

Problem: x[4,2048,1024], 16 heads (d=64), fp32. out = softmax(QK^T/sqrt(d) + mask) V @ Wo.

Sharding: core = (batch b in 0..3) x (head-group hg in 0..1). Each core handles one
batch element and 8 heads (a 512-wide slice of the model dim) and emits a partial
output [2048,1024]; the host sums the two head-group partials per batch element.

Per-core pipeline (v2 — ACT/PE balanced, all-bf16 operands, fp32 PSUM accumulate):
  A) Q^T,K^T projections in transposed layout [128, 2048] bf16 (head pairs packed
     along partitions); V natural, head-interleaved with a ones column per head
     ([128, 8*65] bf16). 1/sqrt(d) is folded into Wq host-side. x and all weights
     are pre-converted to bf16 on the host (halves DMA + SBUF).
  B) per (query-block qb of 512, head): logits in key-major layout [128 k, 512 q]
     (contraction 64), two key-chunks per [128,1024] PSUM tile; ONE wide ACT exp
     -> bf16 probabilities in SBUF (per-key mask bias as per-partition activation
     bias; all-ones masks let both chunks share the zero bias; general masks use
     single-chunk groups); natural-layout AV: acc[128 q, 65] += pt_chunk^T @ V_aug
     (65-row instructions) accumulated over all 16 key chunks; the ones column
     yields the softmax row sum in col 64. Normalize = one DVE reciprocal [128,4]
     + one DVE broadcast-multiply to bf16; PE-transpose per 128-query chunk back
     to [64 d, 128 q], packing head pairs into attT tiles [128, 512].
  C) out[qc, n] = attT-chunk^T @ Wo (K=512 accumulation in 4 chunks), DVE copy,
     DMA out.

Scheduling: the attention loop is ACT(exp)-bound; projections beyond a tiny
prefix plus all stage C work are interleaved into the PE slack between logits
groups via a row-credit scheduler, and AV groups trail their exp by two groups so
the PE never blocks on the ACT engine. Inputs arrive as ONE strided DMA per
tensor (HWDGE issue cost dominates small transfers), ordered so the first
logits group can start ~5us in.
"""
import sys

sys.path.insert(0, "/opt/trn_rl_repo")

from collections import deque

import numpy as np

import concourse.bass as bass
import concourse.tile as tile
from concourse import mybir
from concourse.bass_utils import run_bass_kernel_spmd
from concourse.vector_clock import ScopedClock

_wsplit_ctr = [0]


def split_multi_waits(nc):
    """The walrus build in this container accepts at most ONE sync wait per
    instruction. Split any instruction carrying N>1 waits into (N-1)
    single-wait nops on the same engine immediately before it; the original
    instruction keeps one wait and all its updates."""
    for f in nc.m.functions:
        for bb in f.blocks:
            out = []
            changed = False
            for inst in bb.instructions:
                si = inst.sync_info
                waits = list(si.on_wait) if si is not None and si.on_wait else []
                if len(waits) > 1:
                    updates = list(si.on_update) if si.on_update else []
                    for w in waits[1:]:
                        _wsplit_ctr[0] += 1
                        nop = mybir.InstNoOp(
                            name=f"I-wsplit-{_wsplit_ctr[0]}", ins=[], outs=[]
                        )
                        nop.engine = inst.engine
                        nop.sync_info = mybir.SyncInfo(on_wait=[w], on_update=[])
                        out.append(nop)
                    inst.sync_info = mybir.SyncInfo(on_wait=[waits[0]], on_update=updates)
                    changed = True
                out.append(inst)
            if changed:
                bb.instructions = out
    return nc

B, S, D, H, DH = 4, 2048, 1024, 16, 64
HG = 2  # head groups (tensor-parallel)
LD = D // HG  # 512 local model-dim slice
LH = H // HG  # 8 local heads
N_CORES = B * HG
SCALE = float(DH) ** -0.5
NEG_INF = -1e30

FP = mybir.dt.float32
BF = mybir.dt.bfloat16
F8 = mybir.dt.float8e4
DR = mybir.MatmulPerfMode.DoubleRow
PRE = 32.0  # weight pre-scale so fp8 values sit in e4m3's normal range

KC = D // 128  # 8 contraction chunks (projections)
MC = LD // 128  # 4 head pairs
SC = S // 128  # 16 key chunks of 128
QB = S // 512  # 4 query blocks of 512
Exp = mybir.ActivationFunctionType.Exp
E1 = DH + 1  # per-head V stride incl. ones column


class SplitDrainTileContext(tile.TileContext):
    """The walrus build in this container rejects a Drain instruction with
    more than one sync wait; gate the tail drain with single-wait nops."""

    def _drain_and_barrier(self, tick_clock, wait_clock):
        nc = self.nc
        probe = nc.sync.nop()
        wait_clock.add_sem_waits(
            probe.ins, ScopedClock({None: tick_clock.global_clock})
        )
        si = probe.ins.sync_info
        waits = list(si.on_wait) if si is not None and si.on_wait else []
        updates = list(si.on_update) if si is not None and si.on_update else []
        if len(waits) > 1:
            probe.ins.sync_info = mybir.SyncInfo(on_wait=[waits[0]], on_update=updates)
            for w in waits[1:]:
                n2 = nc.sync.nop()
                n2.ins.sync_info = mybir.SyncInfo(on_wait=[w], on_update=[])
        nc.sync.drain()
        nc.all_engine_barrier()
        popped = nc._tile_sem_poison_stack.pop()
        assert popped is self._sem_poison
        nc.clear_and_free_semaphores(list(self.sems.allocated().values()))
        nc.all_engine_barrier()


def build_nc(for_hw=True, ones_mask=True):
    nc = bass.Bass(trn_type="TRN2")
    x8h = nc.dram_tensor("x8h", [D, S], F8, kind="ExternalInput").ap()
    x8l = nc.dram_tensor("x8l", [D, S], F8, kind="ExternalInput").ap()
    wq8 = [nc.dram_tensor(f"wq8{t}", [D, LD], F8, kind="ExternalInput").ap() for t in "hl"]
    wk8 = [nc.dram_tensor(f"wk8{t}", [D, LD], F8, kind="ExternalInput").ap() for t in "hl"]
    wv8 = [nc.dram_tensor(f"wv8{t}", [D, LD], F8, kind="ExternalInput").ap() for t in "hl"]
    wo = nc.dram_tensor("wo", [LD, D], BF, kind="ExternalInput").ap()
    kbias = nc.dram_tensor("kbias", [128, SC], FP, kind="ExternalInput").ap()
    ident = nc.dram_tensor("ident", [128, 128], BF, kind="ExternalInput").ap()
    ones_b = nc.dram_tensor("ones_b", [128, 128], BF, kind="ExternalInput").ap()
    out = nc.dram_tensor("out", [S, D], FP, kind="ExternalOutput").ap()

    with SplitDrainTileContext(nc) as tc:
        _body(tc, x8h, x8l, wq8, wk8, wv8, wo, kbias, ident, ones_b, out, ones_mask)
    if for_hw:
        split_multi_waits(nc)
    return nc


def _body(tc, x8h, x8l, wq8, wk8, wv8, wo, kbias, ident, ones_b, out, ones_mask):
    nc = tc.nc
    GSZ = 2 if ones_mask else 1  # key chunks per exp group
    NG = SC // GSZ
    W = GSZ * 512

    with (
        tc.tile_pool(name="pers", bufs=1) as pers,
        tc.tile_pool(name="pt", bufs=(SC // (2 if ones_mask else 1)) + 3) as pt_pool,
        tc.tile_pool(name="an", bufs=2) as an_pool,
        tc.tile_pool(name="rc", bufs=2) as rc_pool,
        tc.tile_pool(name="ot", bufs=4) as ot_pool,
        tc.tile_pool(name="att", bufs=2) as att_pool,
        tc.tile_pool(name="ps_lg", bufs=1, space="PSUM") as ps_lg,
        tc.tile_pool(name="ps_pj", bufs=1, space="PSUM") as ps_pj,
    ):
        # ---- tiles (hi/lo fp8 pairs for the compensated projections) ----
        xsl = [
            [
                pers.tile([128, KC * 512], F8, tag=f"x{sl}{t}", name=f"x{sl}{t}")
                for t in "hl"
            ]
            for sl in range(4)
        ]
        wk_m0 = [pers.tile([128, KC * 128], F8, tag=f"wkm0{t}", name=f"wkm0{t}") for t in "hl"]
        wq_m0 = [pers.tile([128, KC * 128], F8, tag=f"wqm0{t}", name=f"wqm0{t}") for t in "hl"]
        wk_m123 = [pers.tile([128, KC * 384], F8, tag=f"wkm1{t}", name=f"wkm1{t}") for t in "hl"]
        wq_m123 = [pers.tile([128, KC * 384], F8, tag=f"wqm1{t}", name=f"wqm1{t}") for t in "hl"]
        wvt = [pers.tile([128, KC * 512], F8, tag=f"wvt{t}", name=f"wvt{t}") for t in "hl"]
        qt = [pers.tile([128, S], BF, tag=f"qt{m}", name=f"qt{m}") for m in range(MC)]
        kt = [pers.tile([128, S], BF, tag=f"kt{m}", name=f"kt{m}") for m in range(MC)]
        vtb = pers.tile([128, SC * LH * E1], BF, tag="vtb")
        vt = [vtb[:, s * LH * E1 : (s + 1) * LH * E1] for s in range(SC)]
        wos = pers.tile([128, MC * D], BF, tag="wos")
        idt = pers.tile([128, 128], BF, tag="idt")
        biasT = pers.tile([128, SC], FP, tag="biasT")

        onesb = pers.tile([128, 8], BF, tag="onesb")

        # ---- DMA (one strided transfer per tensor; order = arrival order) ----
        def dma3(dst, src_rows, k_count, cols, col_lo=0):
            """dst [128, k_count*cols] <- src rows k*128+p, cols col_lo:+cols"""
            d3 = dst[:].rearrange("p (k c) -> p k c", k=k_count)
            s3 = (
                src_rows.rearrange("(k p) c -> p k c", p=128)[
                    :, 0:k_count, col_lo : col_lo + cols
                ]
            )
            nc.sync.dma_start(d3, s3)

        nc.sync.dma_start(onesb[:], ones_b[:, 0:8])
        nc.sync.dma_start(biasT[:], kbias[:])
        x8 = [x8h, x8l]
        for t in range(2):
            dma3(xsl[0][t], x8[t][:, 0:512], KC, 512)
        for t in range(2):
            dma3(wk_m0[t], wk8[t], KC, 128)
        for t in range(2):
            dma3(wq_m0[t], wq8[t], KC, 128)
        nc.sync.dma_start(idt[:], ident[:])
        for t in range(2):
            dma3(wvt[t], wv8[t], KC, 512)
        for sl in range(1, 4):
            for t in range(2):
                dma3(xsl[sl][t], x8[t][:, sl * 512 : (sl + 1) * 512], KC, 512)
        for t in range(2):
            dma3(wk_m123[t], wk8[t], KC, 384, 128)
        for t in range(2):
            dma3(wq_m123[t], wq8[t], KC, 384, 128)
        dma3(wos, wo, MC, D)
        # ones columns of V_aug: one broadcast DVE copy (a strided DMA would
        # cost 16K descriptors on the DMA engines)
        vt_ones = vtb[:].rearrange("p (s h e) -> p s h e", s=SC, e=E1)[
            :, :, :, DH : DH + 1
        ]
        nc.vector.tensor_copy(
            vt_ones,
            onesb[:, 0:8].unsqueeze(1).unsqueeze(3).broadcast_to((128, SC, LH, 1)),
        )

        # ---- PSUM layout (8 banks) ----
        # A matmul with start=True marks its whole 2KB bank row pending-zero,
        # so every accumulation group must own its bank exclusively:
        #   lg ring 2 x [128,W<=1024] fp32 (bank-aligned 512-col groups) = 4
        #   scr [128,512] fp32 = 1 bank: AV accumulator (4 qc groups share ONE
        #     start=True per unit; its pending-zero covers all of them)
        #   shared ring tag "psv" 3 x 1 bank: projection groups, transpose
        #     outputs, stage C groups (each slot bank-exclusive)
        scr = ps_lg.tile([128, 512], FP, tag="scr", name="scr", bufs=1)
        acc = scr[:, 0:260]
        acc3 = acc.rearrange("p (q e) -> p q e", e=E1)

        # ---- projection granules: 3-term compensated fp8 DoubleRow ----
        # lhsT/rhs are [128, 2, M] views (two contraction k-tiles per instr);
        # terms hi@hi + hi@lo + lo@hi accumulate in one psum group, giving
        # ~bf16 accuracy at 0.5 cycles/row.
        def dr3(ps, lhs_pair, rhs_pair, p, last_p):
            terms = ((0, 0), (0, 1), (1, 0))
            for t, (a, b) in enumerate(terms):
                nc.tensor.matmul(
                    ps,
                    lhs_pair[a](p),
                    rhs_pair[b](p),
                    start=(p == 0 and t == 0),
                    stop=(p == KC // 2 - 1 and t == 2),
                    perf_mode=DR,
                    skip_group_check=True,
                )

        def x_view(sl, t, lo, w):
            v = xsl[sl][t][:].rearrange("p (k c) -> p k c", k=KC)
            return lambda p: v[:, 2 * p : 2 * p + 2, lo : lo + w]

        def w_view(tiles, t, m):
            if m == 0:
                v = tiles[0][t][:].rearrange("p (k c) -> p k c", k=KC)
                return lambda p: v[:, 2 * p : 2 * p + 2, :]
            v = tiles[1][t][:].rearrange("p (k c) -> p k c", k=KC)
            return lambda p: v[:, 2 * p : 2 * p + 2, (m - 1) * 128 : m * 128]


        def wv_view(t, m):
            v = wvt[t][:].rearrange("p (k c) -> p k c", k=KC)
            return lambda p: v[:, 2 * p : 2 * p + 2, m * 128 : (m + 1) * 128]

        def v_granule(s_idx, m):
            # one 128-key chunk of V for one head pair (2 heads + ones cols)
            sl, sc4 = s_idx // 4, s_idx % 4
            ps = ps_pj.tile([128, 512], FP, tag="psv", name="psv", bufs=3)
            lhs = (x_view(sl, 0, sc4 * 128, 128), x_view(sl, 1, sc4 * 128, 128))
            rhs = (wv_view(0, m), wv_view(1, m))
            for p in range(KC // 2):
                dr3(ps[:, 0:128], lhs, rhs, p, KC // 2 - 1)
            src = ps[:, 0:128].rearrange("p (h e) -> p h e", h=2)
            dst = vt[s_idx].rearrange("p (h e) -> p h e", e=E1)[
                :, 2 * m : 2 * m + 2, 0:DH
            ]
            nc.vector.tensor_copy(dst, src)

        def k_granule(kc, m):
            # one 128-key chunk of K^T for head pair m
            sl, sc4 = kc // 4, kc % 4
            ps = ps_pj.tile([128, 512], FP, tag="psv", name="psk", bufs=3)
            lhs = (w_view((wk_m0, wk_m123), 0, m), w_view((wk_m0, wk_m123), 1, m))
            rhs = (x_view(sl, 0, sc4 * 128, 128), x_view(sl, 1, sc4 * 128, 128))
            for p in range(KC // 2):
                dr3(ps[:, 0:128], lhs, rhs, p, KC // 2 - 1)
            nc.vector.tensor_copy(kt[m][:, kc * 128 : (kc + 1) * 128], ps[:, 0:128])

        def q_granule(qb, m):
            # a full 512-query block of Q^T for head pair m
            ps = ps_pj.tile([128, 512], FP, tag="psv", name="psq", bufs=3)
            lhs = (w_view((wq_m0, wq_m123), 0, m), w_view((wq_m0, wq_m123), 1, m))
            rhs = (x_view(qb, 0, 0, 512), x_view(qb, 1, 0, 512))
            for p in range(KC // 2):
                dr3(ps[:], lhs, rhs, p, KC // 2 - 1)
            nc.vector.tensor_copy(qt[m][:, qb * 512 : (qb + 1) * 512], ps[:])

        # ---- stage B pieces ----
        def emit_lg_exp(m, h, qb, gi):
            hs = slice(h * 64, (h + 1) * 64)
            qs = slice(qb * 512, (qb + 1) * 512)
            lg = ps_lg.tile([128, W], FP, tag="lg", name="lg", bufs=2)
            for j in range(GSZ):
                kc = gi * GSZ + j
                ks = slice(kc * 128, (kc + 1) * 128)
                nc.tensor.matmul(
                    lg[:, j * 512 : (j + 1) * 512],
                    kt[m][hs, ks],
                    qt[m][hs, qs],
                    start=True,
                    stop=True,
                )
            pt = pt_pool.tile([128, W], BF, tag="pt", name="pt")
            # all-ones mask: kbias is all zeros so one bias column serves both
            # key chunks; general masks use GSZ=1.
            nc.scalar.activation(
                pt[:],
                lg[:],
                Exp,
                bias=biasT[:, gi * GSZ : gi * GSZ + 1],
                scale=1.0 / (PRE * PRE),
            )
            return pt

        def emit_av(pt, m, h, gi):
            for j in range(GSZ):
                kc = gi * GSZ + j
                for qc in range(4):
                    nc.tensor.matmul(
                        acc[:, qc * E1 : (qc + 1) * E1],
                        pt[:, j * 512 + qc * 128 : j * 512 + (qc + 1) * 128],
                        vt[kc][:, (2 * m + h) * E1 : (2 * m + h + 1) * E1],
                        # ONE start per unit: it marks the whole acc bank
                        # pending-zero, which also zero-initializes the other
                        # qc groups' first write
                        start=(kc == 0 and qc == 0),
                        stop=(kc == SC - 1),
                        skip_group_check=True,
                    )

        def unit_end_dve(an_pair, h):
            # normalize this head's attn rows into its interleaved half of the
            # pair's an tile: cols qc*128 + h*64 .. +64
            rct = rc_pool.tile([128, 4], FP, tag="rc", name="rc")
            with nc.allow_low_precision(reason="softmax reciprocal"):
                nc.vector.reciprocal(rct[:].unsqueeze(2), acc3[:, :, DH : DH + 1])
            with nc.allow_low_precision(reason="bf16 attn weights"):
                nc.vector.tensor_tensor(
                    out=an_pair[:].rearrange("p (q c) -> p q c", c=128)[
                        :, :, h * 64 : (h + 1) * 64
                    ],
                    in0=acc3[:, :, 0:DH],
                    in1=rct[:].unsqueeze(2).broadcast_to((128, 4, DH)),
                    op=mybir.AluOpType.mult,
                )

        def unit_end_pe(an_pair, att_cur, m):
            # both heads at once: [128 q, 128 (h d)] -> [128 (h d), 128 q]
            for qc in range(4):
                tp = ps_pj.tile([128, 128], BF, tag="psv", name="tp", bufs=3)[:]
                nc.tensor.matmul(
                    tp,
                    an_pair[:, qc * 128 : (qc + 1) * 128],
                    idt[:],
                    start=True,
                    stop=True,
                    is_transpose=True,
                )
                nc.vector.tensor_copy(att_cur[m][:, qc * 128 : (qc + 1) * 128], tp)

        def stage_c_granule(att_cur, qb, qc, n):
            half = ps_pj.tile([128, 256], FP, tag="psv", name="psc", bufs=3)[:]
            for j in range(MC):
                nc.tensor.matmul(
                    half,
                    att_cur[j][:, qc * 128 : (qc + 1) * 128],
                    wos[:, j * D + n * 256 : j * D + (n + 1) * 256],
                    start=(j == 0),
                    stop=(j == MC - 1),
                )
            ot = ot_pool.tile([128, 256], FP, tag="ot", name="ot")
            nc.vector.tensor_copy(ot[:], half)
            r0 = qb * 512 + qc * 128
            nc.sync.dma_start(out[r0 : r0 + 128, n * 256 : (n + 1) * 256], ot[:])

        # ---- scheduler ----
        emitted = set()
        credit = [0.0]
        CREDIT_CAP = 6000.0  # rows; bounds aux bursts so ACT never starves
        proj_q = deque()  # projection granules (push order)
        c_q = deque()  # stage C granules (drained with priority)

        def emit_V(kc, m):
            if ("V", kc, m) in emitted:
                return 0
            emitted.add(("V", kc, m))
            v_granule(kc, m)
            credit[0] -= KC * 128 * 3 // 4
            return 0

        def emit_K(kc, m):
            if ("K", kc, m) in emitted:
                return 0
            emitted.add(("K", kc, m))
            k_granule(kc, m)
            credit[0] -= KC * 128 * 3 // 4
            return 0

        def emit_Q(qb, m):
            if ("Q", qb, m) in emitted:
                return 0
            emitted.add(("Q", qb, m))
            q_granule(qb, m)
            credit[0] -= KC * 512 * 3 // 4
            return 0

        def push_aux():
            while (c_q or proj_q) and credit[0] > 0:
                q = c_q if c_q else proj_q
                credit[0] -= q.popleft()()

        # Unit-pair order: interleave projection-heavy qb0 pairs with
        # projection-light qb1 pairs so the PE prefix deficit is spread; all
        # projections are done by the end of qb0's last pair.
        ORDER = [
            (0, 0), (1, 0), (0, 1), (1, 1), (0, 2), (1, 2), (0, 3), (1, 3),
            (2, 0), (2, 1), (2, 2), (2, 3), (3, 0), (3, 1), (3, 2), (3, 3),
        ]

        # push order follows the unit order: Q for upcoming pairs, then that
        # pair's K and V chunks (pulls cover anything the pushes miss)
        for qb, m in ORDER:
            if (qb, m) != (0, 0):
                proj_q.append(lambda qb=qb, m=m: emit_Q(qb, m))
            if qb == 0:
                for kc in range(SC):
                    proj_q.append(lambda kc=kc, m=m: emit_K(kc, m))
                for kc in range(SC):
                    proj_q.append(lambda kc=kc, m=m: emit_V(kc, m))

        # prefix: minimal work before the first logits group
        for kc in range(GSZ):
            emit_K(kc, 0)
        emit_Q(0, 0)

        ACT_NS_PER_GROUP = W * 0.8333 + 185.0
        PE_NS_PER_ROW = 1.0 / 2.4
        LAG = NG  # AV trails exp by a full unit: exp runs a unit ahead of AV

        avq = deque()  # (av_closure, finish_closure_or_None)
        pend_pe = [None]

        def pop_av():
            av, finish = avq.popleft()
            av()
            if finish is not None:
                pend_pe[0] = finish()

        att_by_qb = {}
        for qb, m in ORDER:
            if qb not in att_by_qb:
                # att ring has 3 slots; slot reuse at qb needs stage C of qb-3
                # fully emitted first
                for fn in [
                    f for f in list(c_q) if getattr(f, "cqb", -1) == qb - 3
                ]:
                    c_q.remove(fn)
                    fn()
                att_by_qb[qb] = [
                    att_pool.tile(
                        [128, 512], BF, tag=f"at{mm}", name=f"at{mm}", bufs=3
                    )
                    for mm in range(MC)
                ]
            att_cur = att_by_qb[qb]
            an_pair = an_pool.tile([128, 512], BF, tag="an", name="an")
            emit_Q(qb, m)
            for h in range(2):
                for gi in range(NG):
                    for j in range(GSZ):
                        emit_K(gi * GSZ + j, m)
                    pt = emit_lg_exp(m, h, qb, gi)
                    if pend_pe[0] is not None:
                        pend_pe[0]()
                        pend_pe[0] = None
                    if len(avq) >= LAG:
                        pop_av()

                    def mk_av(pt=pt, m=m, h=h, gi=gi):
                        for j in range(GSZ):
                            emit_V(gi * GSZ + j, m)
                        emit_av(pt, m, h, gi)

                    if gi == NG - 1:

                        def mk_finish(
                            an_pair=an_pair, att_cur=att_cur, m=m, h=h, qb=qb
                        ):
                            unit_end_dve(an_pair, h)
                            if h != 1:
                                return None

                            def pe_fin():
                                unit_end_pe(an_pair, att_cur, m)
                                if m == MC - 1:
                                    # att for this qb complete: queue stage C
                                    for qc in range(4):
                                        for n in range(4):

                                            def cfn(
                                                att_cur=att_cur, qb=qb, qc=qc, n=n
                                            ):
                                                stage_c_granule(att_cur, qb, qc, n)
                                                return MC * 256

                                            cfn.cqb = qb
                                            c_q.append(cfn)

                            return pe_fin

                        avq.append((mk_av, mk_finish))
                    else:
                        avq.append((mk_av, None))
                    credit[0] += ACT_NS_PER_GROUP / PE_NS_PER_ROW - (
                        W + GSZ * 4 * E1
                    )
                    credit[0] = min(credit[0], CREDIT_CAP)
                    push_aux()
        # drain: alternate AV pops with leftover stage C to overlap the tail
        while avq:
            pop_av()
            if pend_pe[0] is not None:
                pend_pe[0]()
                pend_pe[0] = None
            for _ in range(2):
                if c_q:
                    c_q.popleft()()
        credit[0] = float("inf")
        push_aux()


_ncs = {}


def get_nc(ones_mask=True):
    if ones_mask not in _ncs:
        _ncs[ones_mask] = build_nc(ones_mask=ones_mask)
    return _ncs[ones_mask]


def make_in_maps(x, mask, Wq, Wk, Wv, Wo):
    import ml_dtypes

    bf = ml_dtypes.bfloat16
    f8 = ml_dtypes.float8_e4m3
    x = np.asarray(x, dtype=np.float32)
    mask = np.asarray(mask)
    Wq, Wk, Wv, Wo = (np.asarray(w, dtype=np.float32) for w in (Wq, Wk, Wv, Wo))
    Wq = Wq * SCALE  # fold 1/sqrt(d) into the Q projection

    def split8(a):
        hi = a.astype(f8)
        lo = (a - hi.astype(np.float32)).astype(f8)
        return hi, lo

    in_maps = []
    for c in range(N_CORES):
        b, hg = c // HG, c % HG
        lo_, hi_ = hg * LD, (hg + 1) * LD
        kb = np.where(mask[b], 0.0, NEG_INF).astype(np.float32)
        xh, xl = split8(np.ascontiguousarray(x[b].T))
        wqh, wql = split8(np.ascontiguousarray(Wq[:, lo_:hi_]) * PRE)
        wkh, wkl = split8(np.ascontiguousarray(Wk[:, lo_:hi_]) * PRE)
        wvh, wvl = split8(np.ascontiguousarray(Wv[:, lo_:hi_]) * PRE)
        in_maps.append(
            {
                "x8h": xh,
                "x8l": xl,
                "wq8h": wqh,
                "wq8l": wql,
                "wk8h": wkh,
                "wk8l": wkl,
                "wv8h": wvh,
                "wv8l": wvl,
                "wo": np.ascontiguousarray(Wo[lo_:hi_, :]).astype(bf),
                "kbias": np.ascontiguousarray(kb.reshape(SC, 128).T),
                "ident": np.eye(128, dtype=bf),
                # 32x ones column cancels the x32 weight pre-scale of V in the
                # softmax normalization
                "ones_b": np.full((128, 128), PRE, np.float32).astype(bf),
            }
        )
    return in_maps


def kernel(x, mask, Wq, Wk, Wv, Wo):
    ones_mask = bool(np.asarray(mask).all())
    nc = get_nc(ones_mask)
    in_maps = make_in_maps(x, mask, Wq, Wk, Wv, Wo)
    res = run_bass_kernel_spmd(nc, in_maps, list(range(N_CORES)))
    outs = np.empty((B, S, D), dtype=np.float32)
    for b in range(B):
        outs[b] = res.results[2 * b]["out"] + res.results[2 * b + 1]["out"]
    return outs


# revision 23
# speedup vs baseline: 1.4623x; 1.0507x over previous
"""Multi-head attention forward on 8 Trainium2 NeuronCores.

Problem: x[4,2048,1024], 16 heads (d=64), fp32. out = softmax(QK^T/sqrt(d) + mask) V @ Wo.

Sharding: core = (batch b in 0..3) x (head-group hg in 0..1). Each core handles one
batch element and 8 heads (a 512-wide slice of the model dim) and emits a partial
output [2048,1024]; the host sums the two head-group partials per batch element.

Per-core pipeline (v2 — ACT/PE balanced, all-bf16 operands, fp32 PSUM accumulate):
  A) Q^T,K^T projections in transposed layout [128, 2048] bf16 (head pairs packed
     along partitions); V natural, head-interleaved with a ones column per head
     ([128, 8*65] bf16). 1/sqrt(d) is folded into Wq host-side. x and all weights
     are pre-converted to bf16 on the host (halves DMA + SBUF).
  B) per (query-block qb of 512, head): logits in key-major layout [128 k, 512 q]
     (contraction 64), two key-chunks per [128,1024] PSUM tile; ONE wide ACT exp
     -> bf16 probabilities in SBUF (per-key mask bias as per-partition activation
     bias; all-ones masks let both chunks share the zero bias; general masks use
     single-chunk groups); natural-layout AV: acc[128 q, 65] += pt_chunk^T @ V_aug
     (65-row instructions) accumulated over all 16 key chunks; the ones column
     yields the softmax row sum in col 64. Normalize = one DVE reciprocal [128,4]
     + one DVE broadcast-multiply to bf16; PE-transpose per 128-query chunk back
     to [64 d, 128 q], packing head pairs into attT tiles [128, 512].
  C) out[qc, n] = attT-chunk^T @ Wo (K=512 accumulation in 4 chunks), DVE copy,
     DMA out.

Scheduling: the attention loop is ACT(exp)-bound; projections beyond a tiny
prefix plus all stage C work are interleaved into the PE slack between logits
groups via a row-credit scheduler, and AV groups trail their exp by two groups so
the PE never blocks on the ACT engine. Inputs arrive as ONE strided DMA per
tensor (HWDGE issue cost dominates small transfers), ordered so the first
logits group can start ~5us in.
"""
import sys

sys.path.insert(0, "/opt/trn_rl_repo")

from collections import deque

import numpy as np

import concourse.bass as bass
import concourse.tile as tile
from concourse import mybir
from concourse.bass_utils import run_bass_kernel_spmd
from concourse.vector_clock import ScopedClock

_wsplit_ctr = [0]


def split_multi_waits(nc):
    """The walrus build in this container accepts at most ONE sync wait per
    instruction. Split any instruction carrying N>1 waits into (N-1)
    single-wait nops on the same engine immediately before it; the original
    instruction keeps one wait and all its updates."""
    for f in nc.m.functions:
        for bb in f.blocks:
            out = []
            changed = False
            for inst in bb.instructions:
                si = inst.sync_info
                waits = list(si.on_wait) if si is not None and si.on_wait else []
                if len(waits) > 1:
                    updates = list(si.on_update) if si.on_update else []
                    for w in waits[1:]:
                        _wsplit_ctr[0] += 1
                        nop = mybir.InstNoOp(
                            name=f"I-wsplit-{_wsplit_ctr[0]}", ins=[], outs=[]
                        )
                        nop.engine = inst.engine
                        nop.sync_info = mybir.SyncInfo(on_wait=[w], on_update=[])
                        out.append(nop)
                    inst.sync_info = mybir.SyncInfo(on_wait=[waits[0]], on_update=updates)
                    changed = True
                out.append(inst)
            if changed:
                bb.instructions = out
    return nc

B, S, D, H, DH = 4, 2048, 1024, 16, 64
HG = 2  # head groups (tensor-parallel)
LD = D // HG  # 512 local model-dim slice
LH = H // HG  # 8 local heads
N_CORES = B * HG
SCALE = float(DH) ** -0.5
NEG_INF = -1e30

FP = mybir.dt.float32
BF = mybir.dt.bfloat16
F8 = mybir.dt.float8e4
DR = mybir.MatmulPerfMode.DoubleRow
PRE = 32.0  # weight pre-scale so fp8 values sit in e4m3's normal range

KC = D // 128  # 8 contraction chunks (projections)
MC = LD // 128  # 4 head pairs
SC = S // 128  # 16 key chunks of 128
QB = S // 512  # 4 query blocks of 512
Exp = mybir.ActivationFunctionType.Exp
E1 = DH + 1  # per-head V stride incl. ones column


class SplitDrainTileContext(tile.TileContext):
    """The walrus build in this container rejects a Drain instruction with
    more than one sync wait; gate the tail drain with single-wait nops."""

    def _drain_and_barrier(self, tick_clock, wait_clock):
        nc = self.nc
        probe = nc.sync.nop()
        wait_clock.add_sem_waits(
            probe.ins, ScopedClock({None: tick_clock.global_clock})
        )
        si = probe.ins.sync_info
        waits = list(si.on_wait) if si is not None and si.on_wait else []
        updates = list(si.on_update) if si is not None and si.on_update else []
        if len(waits) > 1:
            probe.ins.sync_info = mybir.SyncInfo(on_wait=[waits[0]], on_update=updates)
            for w in waits[1:]:
                n2 = nc.sync.nop()
                n2.ins.sync_info = mybir.SyncInfo(on_wait=[w], on_update=[])
        nc.sync.drain()
        nc.all_engine_barrier()
        popped = nc._tile_sem_poison_stack.pop()
        assert popped is self._sem_poison
        nc.clear_and_free_semaphores(list(self.sems.allocated().values()))
        nc.all_engine_barrier()


def build_nc(for_hw=True, ones_mask=True):
    nc = bass.Bass(trn_type="TRN2")
    x8h = nc.dram_tensor("x8h", [D, S], F8, kind="ExternalInput").ap()
    x8l = nc.dram_tensor("x8l", [D, S], F8, kind="ExternalInput").ap()
    wq8 = [nc.dram_tensor(f"wq8{t}", [D, LD], F8, kind="ExternalInput").ap() for t in "hl"]
    wk8 = [nc.dram_tensor(f"wk8{t}", [D, LD], F8, kind="ExternalInput").ap() for t in "hl"]
    wv8 = [nc.dram_tensor(f"wv8{t}", [D, LD], F8, kind="ExternalInput").ap() for t in "hl"]
    wo = nc.dram_tensor("wo", [LD, D], BF, kind="ExternalInput").ap()
    kbias = nc.dram_tensor("kbias", [128, SC], FP, kind="ExternalInput").ap()
    ident = nc.dram_tensor("ident", [128, 128], BF, kind="ExternalInput").ap()
    ones_b = nc.dram_tensor("ones_b", [128, 128], BF, kind="ExternalInput").ap()
    out = nc.dram_tensor("out", [S, D], FP, kind="ExternalOutput").ap()

    with SplitDrainTileContext(nc) as tc:
        _body(tc, x8h, x8l, wq8, wk8, wv8, wo, kbias, ident, ones_b, out, ones_mask)
    if for_hw:
        split_multi_waits(nc)
    return nc


def _body(tc, x8h, x8l, wq8, wk8, wv8, wo, kbias, ident, ones_b, out, ones_mask):
    nc = tc.nc
    GSZ = 2 if ones_mask else 1  # key chunks per exp group
    NG = SC // GSZ
    W = GSZ * 512

    with (
        tc.tile_pool(name="pers", bufs=1) as pers,
        tc.tile_pool(name="pt", bufs=(SC // (2 if ones_mask else 1)) + 3) as pt_pool,
        tc.tile_pool(name="an", bufs=2) as an_pool,
        tc.tile_pool(name="rc", bufs=2) as rc_pool,
        tc.tile_pool(name="ot", bufs=4) as ot_pool,
        tc.tile_pool(name="att", bufs=2) as att_pool,
        tc.tile_pool(name="ps_lg", bufs=1, space="PSUM") as ps_lg,
        tc.tile_pool(name="ps_pj", bufs=1, space="PSUM") as ps_pj,
    ):
        # ---- tiles (hi/lo fp8 pairs for the compensated projections) ----
        xsl = [
            [
                pers.tile([128, KC * 512], F8, tag=f"x{sl}{t}", name=f"x{sl}{t}")
                for t in "hl"
            ]
            for sl in range(4)
        ]
        wk_m0 = [pers.tile([128, KC * 128], F8, tag=f"wkm0{t}", name=f"wkm0{t}") for t in "hl"]
        wq_m0 = [pers.tile([128, KC * 128], F8, tag=f"wqm0{t}", name=f"wqm0{t}") for t in "hl"]
        wk_m123 = [pers.tile([128, KC * 384], F8, tag=f"wkm1{t}", name=f"wkm1{t}") for t in "hl"]
        wq_m123 = [pers.tile([128, KC * 384], F8, tag=f"wqm1{t}", name=f"wqm1{t}") for t in "hl"]
        wvt = [pers.tile([128, KC * 512], F8, tag=f"wvt{t}", name=f"wvt{t}") for t in "hl"]
        qt = [pers.tile([128, S], BF, tag=f"qt{m}", name=f"qt{m}") for m in range(MC)]
        kt = [pers.tile([128, S], BF, tag=f"kt{m}", name=f"kt{m}") for m in range(MC)]
        vtb = pers.tile([128, SC * LH * E1], BF, tag="vtb")
        vt = [vtb[:, s * LH * E1 : (s + 1) * LH * E1] for s in range(SC)]
        wos = pers.tile([128, MC * D], BF, tag="wos")
        idt = pers.tile([128, 128], BF, tag="idt")
        biasT = pers.tile([128, SC], FP, tag="biasT")

        onesb = pers.tile([128, 8], BF, tag="onesb")

        # ---- DMA (one strided transfer per tensor; order = arrival order) ----
        def dma3(dst, src_rows, k_count, cols, col_lo=0):
            """dst [128, k_count*cols] <- src rows k*128+p, cols col_lo:+cols"""
            d3 = dst[:].rearrange("p (k c) -> p k c", k=k_count)
            s3 = (
                src_rows.rearrange("(k p) c -> p k c", p=128)[
                    :, 0:k_count, col_lo : col_lo + cols
                ]
            )
            nc.sync.dma_start(d3, s3)

        nc.sync.dma_start(onesb[:], ones_b[:, 0:8])
        nc.sync.dma_start(biasT[:], kbias[:])
        x8 = [x8h, x8l]
        for t in range(2):
            dma3(xsl[0][t], x8[t][:, 0:512], KC, 512)
        for t in range(2):
            dma3(wk_m0[t], wk8[t], KC, 128)
        for t in range(2):
            dma3(wq_m0[t], wq8[t], KC, 128)
        nc.sync.dma_start(idt[:], ident[:])
        for t in range(2):
            dma3(wvt[t], wv8[t], KC, 512)
        for sl in range(1, 4):
            for t in range(2):
                dma3(xsl[sl][t], x8[t][:, sl * 512 : (sl + 1) * 512], KC, 512)
        for t in range(2):
            dma3(wk_m123[t], wk8[t], KC, 384, 128)
        for t in range(2):
            dma3(wq_m123[t], wq8[t], KC, 384, 128)
        dma3(wos, wo, MC, D)
        # ones columns of V_aug: one broadcast DVE copy (a strided DMA would
        # cost 16K descriptors on the DMA engines)
        vt_ones = vtb[:].rearrange("p (s h e) -> p s h e", s=SC, e=E1)[
            :, :, :, DH : DH + 1
        ]
        nc.vector.tensor_copy(
            vt_ones,
            onesb[:, 0:8].unsqueeze(1).unsqueeze(3).broadcast_to((128, SC, LH, 1)),
        )

        # ---- PSUM layout (8 banks) ----
        # A matmul with start=True marks its whole 2KB bank row pending-zero,
        # so every accumulation group must own its bank exclusively:
        #   lg ring 2 x [128,W<=1024] fp32 (bank-aligned 512-col groups) = 4
        #   scr [128,512] fp32 = 1 bank: AV accumulator (4 qc groups share ONE
        #     start=True per unit; its pending-zero covers all of them)
        #   shared ring tag "psv" 3 x 1 bank: projection groups, transpose
        #     outputs, stage C groups (each slot bank-exclusive)
        scr = ps_lg.tile([128, 512], FP, tag="scr", name="scr", bufs=1)
        acc = scr[:, 0:260]
        acc3 = acc.rearrange("p (q e) -> p q e", e=E1)

        # ---- projection granules: 3-term compensated fp8 DoubleRow ----
        # lhsT/rhs are [128, 2, M] views (two contraction k-tiles per instr);
        # terms hi@hi + hi@lo + lo@hi accumulate in one psum group, giving
        # ~bf16 accuracy at 0.5 cycles/row.
        def dr3(ps, lhs_pair, rhs_pair, p, last_p):
            terms = ((0, 0), (0, 1), (1, 0))
            for t, (a, b) in enumerate(terms):
                nc.tensor.matmul(
                    ps,
                    lhs_pair[a](p),
                    rhs_pair[b](p),
                    start=(p == 0 and t == 0),
                    stop=(p == KC // 2 - 1 and t == 2),
                    perf_mode=DR,
                    skip_group_check=True,
                )

        def x_view(sl, t, lo, w):
            v = xsl[sl][t][:].rearrange("p (k c) -> p k c", k=KC)
            return lambda p: v[:, 2 * p : 2 * p + 2, lo : lo + w]

        def w_view(tiles, t, m):
            if m == 0:
                v = tiles[0][t][:].rearrange("p (k c) -> p k c", k=KC)
                return lambda p: v[:, 2 * p : 2 * p + 2, :]
            v = tiles[1][t][:].rearrange("p (k c) -> p k c", k=KC)
            return lambda p: v[:, 2 * p : 2 * p + 2, (m - 1) * 128 : m * 128]


        def wv_view(t, m):
            v = wvt[t][:].rearrange("p (k c) -> p k c", k=KC)
            return lambda p: v[:, 2 * p : 2 * p + 2, m * 128 : (m + 1) * 128]

        def v_granule(s_idx, m):
            # one 128-key chunk of V for one head pair (2 heads + ones cols)
            sl, sc4 = s_idx // 4, s_idx % 4
            ps = ps_pj.tile([128, 512], FP, tag="psv", name="psv", bufs=3)
            lhs = (x_view(sl, 0, sc4 * 128, 128), x_view(sl, 1, sc4 * 128, 128))
            rhs = (wv_view(0, m), wv_view(1, m))
            for p in range(KC // 2):
                dr3(ps[:, 0:128], lhs, rhs, p, KC // 2 - 1)
            src = ps[:, 0:128].rearrange("p (h e) -> p h e", h=2)
            dst = vt[s_idx].rearrange("p (h e) -> p h e", e=E1)[
                :, 2 * m : 2 * m + 2, 0:DH
            ]
            nc.vector.tensor_copy(dst, src)

        def k_granule(kc, m):
            # one 128-key chunk of K^T for head pair m
            sl, sc4 = kc // 4, kc % 4
            ps = ps_pj.tile([128, 512], FP, tag="psv", name="psk", bufs=3)
            lhs = (w_view((wk_m0, wk_m123), 0, m), w_view((wk_m0, wk_m123), 1, m))
            rhs = (x_view(sl, 0, sc4 * 128, 128), x_view(sl, 1, sc4 * 128, 128))
            for p in range(KC // 2):
                dr3(ps[:, 0:128], lhs, rhs, p, KC // 2 - 1)
            nc.vector.tensor_copy(kt[m][:, kc * 128 : (kc + 1) * 128], ps[:, 0:128])

        def q_granule(qb, m):
            # a full 512-query block of Q^T for head pair m
            ps = ps_pj.tile([128, 512], FP, tag="psv", name="psq", bufs=3)
            lhs = (w_view((wq_m0, wq_m123), 0, m), w_view((wq_m0, wq_m123), 1, m))
            rhs = (x_view(qb, 0, 0, 512), x_view(qb, 1, 0, 512))
            for p in range(KC // 2):
                dr3(ps[:], lhs, rhs, p, KC // 2 - 1)
            nc.vector.tensor_copy(qt[m][:, qb * 512 : (qb + 1) * 512], ps[:])

        # ---- stage B pieces ----
        def emit_lg_exp(m, h, qb, gi):
            hs = slice(h * 64, (h + 1) * 64)
            qs = slice(qb * 512, (qb + 1) * 512)
            lg = ps_lg.tile([128, W], FP, tag="lg", name="lg", bufs=2)
            for j in range(GSZ):
                kc = gi * GSZ + j
                ks = slice(kc * 128, (kc + 1) * 128)
                nc.tensor.matmul(
                    lg[:, j * 512 : (j + 1) * 512],
                    kt[m][hs, ks],
                    qt[m][hs, qs],
                    start=True,
                    stop=True,
                )
            pt = pt_pool.tile([128, W], BF, tag="pt", name="pt")
            # all-ones mask: kbias is all zeros so one bias column serves both
            # key chunks; general masks use GSZ=1.
            nc.scalar.activation(
                pt[:],
                lg[:],
                Exp,
                bias=biasT[:, gi * GSZ : gi * GSZ + 1],
                scale=1.0 / (PRE * PRE),
            )
            return pt

        def emit_av(pt, m, h, gi):
            for j in range(GSZ):
                kc = gi * GSZ + j
                for qc in range(4):
                    nc.tensor.matmul(
                        acc[:, qc * E1 : (qc + 1) * E1],
                        pt[:, j * 512 + qc * 128 : j * 512 + (qc + 1) * 128],
                        vt[kc][:, (2 * m + h) * E1 : (2 * m + h + 1) * E1],
                        # ONE start per unit: it marks the whole acc bank
                        # pending-zero, which also zero-initializes the other
                        # qc groups' first write
                        start=(kc == 0 and qc == 0),
                        stop=(kc == SC - 1),
                        skip_group_check=True,
                    )

        def unit_end_dve(an_pair, h):
            # normalize this head's attn rows into its interleaved half of the
            # pair's an tile: cols qc*128 + h*64 .. +64
            rct = rc_pool.tile([128, 4], FP, tag="rc", name="rc")
            with nc.allow_low_precision(reason="softmax reciprocal"):
                nc.vector.reciprocal(rct[:].unsqueeze(2), acc3[:, :, DH : DH + 1])
            with nc.allow_low_precision(reason="bf16 attn weights"):
                nc.vector.tensor_tensor(
                    out=an_pair[:].rearrange("p (q c) -> p q c", c=128)[
                        :, :, h * 64 : (h + 1) * 64
                    ],
                    in0=acc3[:, :, 0:DH],
                    in1=rct[:].unsqueeze(2).broadcast_to((128, 4, DH)),
                    op=mybir.AluOpType.mult,
                )

        def unit_end_pe(an_pair, att_cur, m):
            # both heads at once: [128 q, 128 (h d)] -> [128 (h d), 128 q]
            for qc in range(4):
                tp = ps_pj.tile([128, 128], BF, tag="psv", name="tp", bufs=3)[:]
                nc.tensor.matmul(
                    tp,
                    an_pair[:, qc * 128 : (qc + 1) * 128],
                    idt[:],
                    start=True,
                    stop=True,
                    is_transpose=True,
                )
                nc.vector.tensor_copy(att_cur[m][:, qc * 128 : (qc + 1) * 128], tp)

        def stage_c_granule(att_cur, qb, qc, n):
            half = ps_pj.tile([128, 256], FP, tag="psv", name="psc", bufs=3)[:]
            for j in range(MC):
                nc.tensor.matmul(
                    half,
                    att_cur[j][:, qc * 128 : (qc + 1) * 128],
                    wos[:, j * D + n * 256 : j * D + (n + 1) * 256],
                    start=(j == 0),
                    stop=(j == MC - 1),
                )
            ot = ot_pool.tile([128, 256], FP, tag="ot", name="ot")
            nc.vector.tensor_copy(ot[:], half)
            r0 = qb * 512 + qc * 128
            nc.sync.dma_start(out[r0 : r0 + 128, n * 256 : (n + 1) * 256], ot[:])

        # ---- scheduler ----
        emitted = set()
        credit = [0.0]
        CREDIT_CAP = 6000.0  # rows; bounds aux bursts so ACT never starves
        proj_q = deque()  # projection granules (push order)
        c_q = deque()  # stage C granules (drained with priority)

        def emit_V(kc, m):
            if ("V", kc, m) in emitted:
                return 0
            emitted.add(("V", kc, m))
            v_granule(kc, m)
            credit[0] -= KC * 128 * 3 // 4
            return 0

        def emit_K(kc, m):
            if ("K", kc, m) in emitted:
                return 0
            emitted.add(("K", kc, m))
            k_granule(kc, m)
            credit[0] -= KC * 128 * 3 // 4
            return 0

        def emit_Q(qb, m):
            if ("Q", qb, m) in emitted:
                return 0
            emitted.add(("Q", qb, m))
            q_granule(qb, m)
            credit[0] -= KC * 512 * 3 // 4
            return 0

        def push_aux():
            while (c_q or proj_q) and credit[0] > 0:
                q = c_q if c_q else proj_q
                credit[0] -= q.popleft()()

        # Unit-pair order: interleave projection-heavy qb0 pairs with
        # projection-light qb1 pairs so the PE prefix deficit is spread; all
        # projections are done by the end of qb0's last pair.
        ORDER = [
            (0, 0), (1, 0), (2, 0), (0, 1), (1, 1), (2, 1), (0, 2), (1, 2),
            (2, 2), (0, 3), (1, 3), (2, 3), (3, 0), (3, 1), (3, 2), (3, 3),
        ]

        # push order follows the unit order: Q for upcoming pairs, then that
        # pair's K and V chunks (pulls cover anything the pushes miss)
        for qb, m in ORDER:
            if (qb, m) != (0, 0):
                proj_q.append(lambda qb=qb, m=m: emit_Q(qb, m))
            if qb == 0:
                for kc in range(SC):
                    proj_q.append(lambda kc=kc, m=m: emit_K(kc, m))
                for kc in range(SC):
                    proj_q.append(lambda kc=kc, m=m: emit_V(kc, m))

        # prefix: minimal work before the first logits group
        for kc in range(GSZ):
            emit_K(kc, 0)
        emit_Q(0, 0)

        ACT_NS_PER_GROUP = W * 0.8333 + 185.0
        PE_NS_PER_ROW = 1.0 / 2.4
        LAG = NG  # AV trails exp by a full unit: exp runs a unit ahead of AV

        avq = deque()  # (av_closure, finish_closure_or_None)
        pend_pe = [None]

        def pop_av():
            av, finish = avq.popleft()
            av()
            if finish is not None:
                pend_pe[0] = finish()

        att_by_qb = {}
        for qb, m in ORDER:
            if qb not in att_by_qb:
                # att ring has 3 slots; slot reuse at qb needs stage C of qb-3
                # fully emitted first
                for fn in [
                    f for f in list(c_q) if getattr(f, "cqb", -1) == qb - 3
                ]:
                    c_q.remove(fn)
                    fn()
                att_by_qb[qb] = [
                    att_pool.tile(
                        [128, 512], BF, tag=f"at{mm}", name=f"at{mm}", bufs=3
                    )
                    for mm in range(MC)
                ]
            att_cur = att_by_qb[qb]
            an_pair = an_pool.tile([128, 512], BF, tag="an", name="an")
            emit_Q(qb, m)
            for h in range(2):
                for gi in range(NG):
                    for j in range(GSZ):
                        emit_K(gi * GSZ + j, m)
                    pt = emit_lg_exp(m, h, qb, gi)
                    if pend_pe[0] is not None:
                        pend_pe[0]()
                        pend_pe[0] = None
                    if len(avq) >= LAG:
                        pop_av()

                    def mk_av(pt=pt, m=m, h=h, gi=gi):
                        for j in range(GSZ):
                            emit_V(gi * GSZ + j, m)
                        emit_av(pt, m, h, gi)

                    if gi == NG - 1:

                        def mk_finish(
                            an_pair=an_pair, att_cur=att_cur, m=m, h=h, qb=qb
                        ):
                            unit_end_dve(an_pair, h)
                            if h != 1:
                                return None

                            def pe_fin():
                                unit_end_pe(an_pair, att_cur, m)
                                if m == MC - 1:
                                    # att for this qb complete: queue stage C
                                    for qc in range(4):
                                        for n in range(4):

                                            def cfn(
                                                att_cur=att_cur, qb=qb, qc=qc, n=n
                                            ):
                                                stage_c_granule(att_cur, qb, qc, n)
                                                return MC * 256

                                            cfn.cqb = qb
                                            c_q.append(cfn)

                            return pe_fin

                        avq.append((mk_av, mk_finish))
                    else:
                        avq.append((mk_av, None))
                    credit[0] += ACT_NS_PER_GROUP / PE_NS_PER_ROW - (
                        W + GSZ * 4 * E1
                    )
                    credit[0] = min(credit[0], CREDIT_CAP)
                    push_aux()
        # drain: alternate AV pops with leftover stage C to overlap the tail
        while avq:
            pop_av()
            if pend_pe[0] is not None:
                pend_pe[0]()
                pend_pe[0] = None
            for _ in range(2):
                if c_q:
                    c_q.popleft()()
        credit[0] = float("inf")
        push_aux()


_ncs = {}


def get_nc(ones_mask=True):
    if ones_mask not in _ncs:
        _ncs[ones_mask] = build_nc(ones_mask=ones_mask)
    return _ncs[ones_mask]


def make_in_maps(x, mask, Wq, Wk, Wv, Wo):
    import ml_dtypes

    bf = ml_dtypes.bfloat16
    f8 = ml_dtypes.float8_e4m3
    x = np.asarray(x, dtype=np.float32)
    mask = np.asarray(mask)
    Wq, Wk, Wv, Wo = (np.asarray(w, dtype=np.float32) for w in (Wq, Wk, Wv, Wo))
    Wq = Wq * SCALE  # fold 1/sqrt(d) into the Q projection

    def split8(a):
        hi = a.astype(f8)
        lo = (a - hi.astype(np.float32)).astype(f8)
        return hi, lo

    in_maps = []
    for c in range(N_CORES):
        b, hg = c // HG, c % HG
        lo_, hi_ = hg * LD, (hg + 1) * LD
        kb = np.where(mask[b], 0.0, NEG_INF).astype(np.float32)
        xh, xl = split8(np.ascontiguousarray(x[b].T))
        wqh, wql = split8(np.ascontiguousarray(Wq[:, lo_:hi_]) * PRE)
        wkh, wkl = split8(np.ascontiguousarray(Wk[:, lo_:hi_]) * PRE)
        wvh, wvl = split8(np.ascontiguousarray(Wv[:, lo_:hi_]) * PRE)
        in_maps.append(
            {
                "x8h": xh,
                "x8l": xl,
                "wq8h": wqh,
                "wq8l": wql,
                "wk8h": wkh,
                "wk8l": wkl,
                "wv8h": wvh,
                "wv8l": wvl,
                "wo": np.ascontiguousarray(Wo[lo_:hi_, :]).astype(bf),
                "kbias": np.ascontiguousarray(kb.reshape(SC, 128).T),
                "ident": np.eye(128, dtype=bf),
                # 32x ones column cancels the x32 weight pre-scale of V in the
                # softmax normalization
                "ones_b": np.full((128, 128), PRE, np.float32).astype(bf),
            }
        )
    return in_maps


def kernel(x, mask, Wq, Wk, Wv, Wo):
    ones_mask = bool(np.asarray(mask).all())
    nc = get_nc(ones_mask)
    in_maps = make_in_maps(x, mask, Wq, Wk, Wv, Wo)
    res = run_bass_kernel_spmd(nc, in_maps, list(range(N_CORES)))
    outs = np.empty((B, S, D), dtype=np.float32)
    for b in range(B):
        outs[b] = res.results[2 * b]["out"] + res.results[2 * b + 1]["out"]
    return outs
